# revision 21
# baseline (speedup 1.0000x reference)
"""Cross-attention kernel for 8 TRN2 NeuronCores.

Sharding: core c -> batch b = c//2, head-group g = c%2 (8 of 16 heads).
Each core computes its batch's attention for its 8 heads plus the
row-sharded slice of the output projection; the host sums the two
partial outputs per batch and adds bo.  No device collectives.

Layouts (per core):
  xT, encT      [1024, 2048]  bf16  (d_model on partitions)
  wq/wk/wv      [1024, 512]   bf16  (f = (head, e) flattened, head-major)
  wo            [512, 1024]   bf16
  QT, KT        [512, r]      bf16  (f on partitions)   = W.T @ xT (+b)
  V (natural)   [2048, 8, 65] bf16  (s on partitions, col 64 = ones)
  scores^T      psum f32 [s_tile=128, 3, r=512] batches = KT_h.T-chunk @ QT_h
                (two heads of a pair run concurrently on PE row groups 0/64)
  expS          bf16 [128, 32, 512], exp(0.125*scores^T) in N=1536 batches
                (no max subtraction: |scores| < ~3)
  attnV         psum [65, 512] = [V_h | 1].T @ expS  (row 64 = softmax denom)
  MH^T          [512, 512-block] bf16 = attnV[0:64] * bcast(1/denom)
  out^T         [1024, 2048] f32 = wo.T @ MH^T

The QT projection for r-block r+1 is computed inside r-block r's head
loop so the PE has ready gap-filler work while ScalarE (the attention
phase bottleneck) grinds through the exps -- this keeps PE dense and the
HAM clock at 2.4 GHz.
"""
import sys
import numpy as np

try:
    import concourse.bass as bass
except ImportError:
    sys.path.insert(0, "/opt/trn_rl_repo")
    import concourse.bass as bass

import ml_dtypes
from contextlib import ExitStack

import concourse.mybir as mybir
from concourse import bacc
from concourse.tile import TileContext
from concourse.bass_utils import run_bass_kernel_spmd

BF16 = ml_dtypes.bfloat16
FP32 = mybir.dt.float32
BF = mybir.dt.bfloat16

B, S, D = 4, 2048, 1024          # batch, seq (q and kv), d_model
H, E = 8, 64                     # heads per core, head dim
F = H * E                        # 512, per-core projection width
KT = 8                           # k tiles over d_model
FT = 4                           # f tiles over F
ST = 16                          # s tiles over S
RB = 512                         # r block (matmul moving dim)
NRB = S // RB                    # 4
EB = 4                           # score-psum banks per exp batch (N=2048)

_CACHE = {}


def _build(debug=False):
    nc = bacc.Bacc()
    xT = nc.declare_dram_parameter("xT", [D, S], BF, isOutput=False)
    encT = nc.declare_dram_parameter("encT", [D, S], BF, isOutput=False)
    wq = nc.declare_dram_parameter("wq", [D, F], BF, isOutput=False)
    wk = nc.declare_dram_parameter("wk", [D, F], BF, isOutput=False)
    wv = nc.declare_dram_parameter("wv", [D, F], BF, isOutput=False)
    wo = nc.declare_dram_parameter("wo", [F, D], BF, isOutput=False)
    bq = nc.declare_dram_parameter("bq", [F, 1], FP32, isOutput=False)
    bk = nc.declare_dram_parameter("bk", [F, 1], FP32, isOutput=False)
    bvb = nc.declare_dram_parameter("bvb", [1, F], BF, isOutput=False)
    out = nc.declare_dram_parameter("out", [D, S], FP32, isOutput=True)
    if debug:
        dqt = nc.declare_dram_parameter("dqt", [128, FT, RB], BF, isOutput=True)
        dkt = nc.declare_dram_parameter("dkt", [128, FT, S], BF, isOutput=True)
        dv = nc.declare_dram_parameter("dv", [128, ST, H, E + 1], BF, isOutput=True)
        dexp = nc.declare_dram_parameter("dexp", [128, ST * 2, RB], BF, isOutput=True)
        drec = nc.declare_dram_parameter("drec", [128, RB], FP32, isOutput=True)
        dmh = nc.declare_dram_parameter("dmh", [128, FT, RB], BF, isOutput=True)

    Exp = mybir.ActivationFunctionType.Exp
    Copy = mybir.ActivationFunctionType.Copy

    with TileContext(nc) as tc, ExitStack() as ctx:
        wp = ctx.enter_context(tc.tile_pool(name="weights", bufs=1))
        ap = ctx.enter_context(tc.tile_pool(name="acts", bufs=1))
        pp = ctx.enter_context(tc.tile_pool(name="psum", bufs=2, space="PSUM"))

        # ---- weights + biases in (gpsimd SWDGE queues; sync HWDGE is kept
        # for latency-sensitive small DMAs + output) ----
        wq_s = wp.tile([128, KT, F], BF, tag="wq")
        wk_s = wp.tile([128, KT, F], BF, tag="wk")
        wv_s = wp.tile([128, KT, F], BF, tag="wv")
        for k in range(KT):
            nc.gpsimd.dma_start(out=wv_s[:, k, :], in_=wv[k * 128:(k + 1) * 128, :])
            nc.gpsimd.dma_start(out=wk_s[:, k, :], in_=wk[k * 128:(k + 1) * 128, :])
        bq_s = wp.tile([128, FT], FP32, tag="bq")
        bk_s = wp.tile([128, FT], FP32, tag="bk")
        for f in range(FT):
            nc.sync.dma_start(out=bq_s[:, f:f + 1], in_=bq[f * 128:(f + 1) * 128, :])
            nc.sync.dma_start(out=bk_s[:, f:f + 1], in_=bk[f * 128:(f + 1) * 128, :])
        bv_bf = wp.tile([1, F], BF, tag="bvb")
        ones_r = wp.tile([1, 128], BF, tag="ones")
        ones_c = wp.tile([128, 1], BF, tag="onesc")
        nc.vector.memset(ones_r, 1.0)
        nc.vector.memset(ones_c, 1.0)

        atp = ctx.enter_context(tc.tile_pool(name="attn", bufs=1))
        dsp = ctx.enter_context(tc.tile_pool(name="dscratch", bufs=2, space="DRAM"))

        def load_enc(r):
            e_r = atp.tile([128, KT, RB], BF, tag="enc", bufs=2)
            for k in range(KT):
                nc.gpsimd.dma_start(
                    out=e_r[:, k, :],
                    in_=encT[k * 128:(k + 1) * 128, r * RB:(r + 1) * RB])
            return e_r

        def load_x(r):
            x_r = atp.tile([128, KT, RB], BF, tag="xs", bufs=2)
            for k in range(KT):
                nc.gpsimd.dma_start(
                    out=x_r[:, k, :],
                    in_=xT[k * 128:(k + 1) * 128, r * RB:(r + 1) * RB])
            return x_r

        def qt_proj(x_r, f, qt_r):
            ps = pp.tile([128, RB], FP32, tag="ps_mm", bufs=1)
            for k in range(KT):
                nc.tensor.matmul(
                    ps, wq_s[:, k, f * 128:(f + 1) * 128], x_r[:, k, :],
                    start=(k == 0), stop=(k == KT - 1))
            nc.vector.tensor_scalar_add(out=qt_r[:, f, :], in0=ps,
                                        scalar1=bq_s[:, f:f + 1])

        nc.sync.dma_start(out=bv_bf, in_=bvb[:])

        # ---- V projection first (feeds attnV; ScalarE does the psum->V
        # copies so it has warm-up work while PE runs dense matmuls) ----
        kt_s = ap.tile([128, FT, S], BF, tag="kts")
        x_cur = load_x(0)
        qt_cur = atp.tile([128, FT, RB], BF, tag="qt", bufs=2)
        v_s = ap.tile([128, ST, H, E + 1], BF, tag="vs")
        nc.vector.memset(v_s[:, :, :, E:E + 1], 1.0)
        for rc in range(NRB):
            e_r = load_enc(rc)
            for sl in range(4):
                s = rc * 4 + sl
                ps = pp.tile([128, RB], FP32, tag="ps_mm", bufs=1)
                nc.tensor.matmul(ps, ones_r[:], bv_bf[:],
                                 start=True, stop=False)
                for k in range(KT):
                    nc.tensor.matmul(
                        ps, e_r[:, k, sl * 128:(sl + 1) * 128], wv_s[:, k, :],
                        start=False, stop=(k == KT - 1))
                nc.scalar.activation(
                    out=v_s[:, s, :, 0:E],
                    in_=ps.rearrange("p (h e) -> p h e", h=H),
                    func=Copy, bias=0.0, scale=1.0)

        # remaining weights (first needed mid r-block 0)
        for k in range(KT):
            nc.gpsimd.dma_start(out=wq_s[:, k, :], in_=wq[k * 128:(k + 1) * 128, :])
        wo_s = wp.tile([128, FT, D], BF, tag="wo")
        for f in range(FT):
            nc.gpsimd.dma_start(out=wo_s[:, f, :], in_=wo[f * 128:(f + 1) * 128, :])

        # ---- attention + output projection, per r block ----
        for r in range(NRB):
            rsl = slice(r * RB, (r + 1) * RB)
            mh_s = atp.tile([128, FT, RB], BF, tag="mh", bufs=2)
            if r + 1 < NRB:
                x_next = load_x(r + 1)
                qt_next = atp.tile([128, FT, RB], BF, tag="qt", bufs=2)
            for hp in range(FT):  # heads 2hp (partitions 0-63), 2hp+1 (64-127)
                if r == 0:
                    # KT projection for f=hp (enc streamed) + QT(0) f=hp,
                    # just ahead of the first scores that need them
                    for rk in range(NRB):
                        e_r = load_enc(rk)
                        ps = pp.tile([128, RB], FP32, tag="ps_mm", bufs=1)
                        for k in range(KT):
                            nc.tensor.matmul(
                                ps, wk_s[:, k, hp * 128:(hp + 1) * 128],
                                e_r[:, k, :],
                                start=(k == 0), stop=(k == KT - 1))
                        nc.vector.tensor_scalar_add(
                            out=kt_s[:, hp, rk * RB:(rk + 1) * RB], in0=ps,
                            scalar1=bk_s[:, hp:hp + 1])
                    qt_proj(x_cur, hp, qt_cur)
                expS = atp.tile([128, ST * 2, RB], BF, tag="expS", bufs=2)
                # scores^T + exp, in EB-bank psum batches over the 32
                # (s_tile, head01) slices
                j = 0
                while j < ST * 2:
                    bn = min(EB, ST * 2 - j)
                    psB = pp.tile([128, EB, RB], FP32, tag="ps_sc", bufs=1)
                    for jj in range(bn):
                        s, hh = (j + jj) // 2, (j + jj) % 2
                        ssl = slice(s * 128, (s + 1) * 128)
                        pr = slice(hh * 64, hh * 64 + 64)
                        nc.tensor.matmul(
                            psB[:, jj, :], kt_s[pr, hp, ssl], qt_cur[pr, hp, :],
                            start=True, stop=True, tile_position=(hh * 64, 0))
                    nc.scalar.activation(out=expS[:, j:j + bn, :],
                                         in_=psB[:, 0:bn, :], func=Exp, scale=0.125)
                    j += bn
                if debug and r == 0 and hp == 0:
                    nc.sync.dma_start(out=dexp[:], in_=expS[:])
                # QT projection for the next r block: PE gap-filler while
                # ScalarE works through the exps of this head pair.
                if r + 1 < NRB:
                    qt_proj(x_next, hp, qt_next)
                h0, h1 = 2 * hp, 2 * hp + 1
                po0 = pp.tile([128, RB], FP32, tag="po0", bufs=1)
                po1 = pp.tile([128, RB], FP32, tag="po1", bufs=1)
                rs = pp.tile([128, RB], FP32, tag="rs", bufs=1)
                for s in range(ST):
                    st, sp = (s == 0), (s == ST - 1)
                    nc.tensor.matmul(
                        po0[0:E, :], v_s[:, s, h0, 0:E], expS[:, 2 * s, :],
                        start=st, stop=sp, tile_position=(0, 0))
                    nc.tensor.matmul(
                        po1[64:128, :], v_s[:, s, h1, 0:E], expS[:, 2 * s + 1, :],
                        start=st, stop=sp, tile_position=(0, 64))
                    nc.tensor.matmul(
                        rs[0:1, :], ones_c[:], expS[:, 2 * s, :],
                        start=st, stop=sp, tile_position=(0, 0))
                    nc.tensor.matmul(
                        rs[32:33, :], ones_c[:], expS[:, 2 * s + 1, :],
                        start=st, stop=sp, tile_position=(0, 32))
                rec = atp.tile([128, RB], FP32, tag="rec", bufs=2)
                rsb = atp.tile([33, RB], FP32, tag="rsb", bufs=2)
                dsc0 = dsp.tile([1, RB], FP32, tag="dsc", bufs=4)
                dsc1 = dsp.tile([1, RB], FP32, tag="dsc", bufs=4)
                nc.vector.tensor_copy(out=rsb[0:1, :], in_=rs[0:1, :])
                nc.vector.tensor_copy(out=rsb[32:33, :], in_=rs[32:33, :])
                nc.sync.dma_start(out=dsc0[:], in_=rsb[0:1, :])
                nc.sync.dma_start(out=dsc1[:], in_=rsb[32:33, :])
                nc.sync.dma_start(out=rec[0:64, :],
                                  in_=dsc0[:].to_broadcast([64, RB]))
                nc.sync.dma_start(out=rec[64:128, :],
                                  in_=dsc1[:].to_broadcast([64, RB]))
                nc.vector.reciprocal_approx_fast(out=rec[:, :], in_=rec[:, :])
                if debug and r == 0 and hp == 0:
                    nc.sync.dma_start(out=drec[:], in_=rec[:])
                nc.vector.tensor_mul(
                    out=mh_s[0:64, hp, :], in0=po0[0:E, :], in1=rec[0:64, :])
                nc.vector.tensor_mul(
                    out=mh_s[64:128, hp, :], in0=po1[64:128, :], in1=rec[64:128, :])
            if debug and r == 0:
                nc.sync.dma_start(out=dmh[:], in_=mh_s[:])
            for dt in range(8):
                pso = pp.tile([128, RB], FP32, tag="ps_mm", bufs=1)
                for f in range(FT):
                    nc.tensor.matmul(
                        pso, wo_s[:, f, dt * 128:(dt + 1) * 128], mh_s[:, f, :],
                        start=(f == 0), stop=(f == FT - 1))
                osb = atp.tile([128, RB], FP32, tag="osb", bufs=3)
                nc.vector.tensor_copy(out=osb, in_=pso)
                nc.sync.dma_start(out=out[dt * 128:(dt + 1) * 128, rsl], in_=osb)
            if r + 1 < NRB:
                x_cur, qt_cur = x_next, qt_next

        if debug:
            nc.sync.dma_start(out=dv[:], in_=v_s[:])

    nc.finalize()
    return nc


def _prep_in_maps(x, enc, Wq, bq, Wk, bk, Wv, bv, Wo):
    def bfc(a):
        return np.ascontiguousarray(a.astype(BF16))

    in_maps = []
    for c in range(8):
        b, g = c // 2, c % 2
        hs = slice(8 * g, 8 * g + 8)
        in_maps.append({
            "xT": bfc(x[b].T),
            "encT": bfc(enc[b].T),
            "wq": bfc(np.transpose(Wq[hs], (1, 0, 2)).reshape(D, F)),
            "wk": bfc(np.transpose(Wk[hs], (1, 0, 2)).reshape(D, F)),
            "wv": bfc(np.transpose(Wv[hs], (1, 0, 2)).reshape(D, F)),
            "wo": bfc(Wo[F * g:F * (g + 1)]),
            "bq": np.ascontiguousarray(bq[hs].reshape(F, 1), dtype=np.float32),
            "bk": np.ascontiguousarray(bk[hs].reshape(F, 1), dtype=np.float32),
            "bvb": bfc(bv[hs].reshape(1, F)),
        })
    return in_maps


def run(inputs, trace=False, debug=False):
    x = np.asarray(inputs["x"], np.float32)
    enc = np.asarray(inputs["encoder_output"], np.float32)
    Wq = np.asarray(inputs["Wq"], np.float32)
    Wk = np.asarray(inputs["Wk"], np.float32)
    Wv = np.asarray(inputs["Wv"], np.float32)
    Wo = np.asarray(inputs["Wo"], np.float32)
    bq = np.asarray(inputs["bq"], np.float32)
    bk = np.asarray(inputs["bk"], np.float32)
    bv = np.asarray(inputs["bv"], np.float32)
    bo = np.asarray(inputs["bo"], np.float32)

    key = "nc_dbg" if debug else "nc"
    if key not in _CACHE:
        _CACHE[key] = _build(debug=debug)
    nc = _CACHE[key]
    in_maps = _prep_in_maps(x, enc, Wq, bq, Wk, bk, Wv, bv, Wo)
    res = run_bass_kernel_spmd(nc, in_maps, core_ids=list(range(8)), trace=trace)
    out = np.zeros((B, S, D), np.float32)
    for b in range(B):
        out[b] = (np.asarray(res.results[2 * b]["out"], np.float32)
                  + np.asarray(res.results[2 * b + 1]["out"], np.float32)).T + bo
    return out, res


def kernel(**inputs):
    out, _ = run(inputs, trace=False)
    return out


# revision 22
# speedup vs baseline: 1.0803x; 1.0803x over previous
"""Cross-attention kernel for 8 TRN2 NeuronCores.

Sharding: core c -> batch b = c//2, head-group g = c%2 (8 of 16 heads).
Each core computes its batch's attention for its 8 heads plus the
row-sharded slice of the output projection; the host sums the two
partial outputs per batch and adds bo.  No device collectives.

Layouts (per core):
  xT, encT      [1024, 2048]  bf16  (d_model on partitions)
  wq/wk/wv      [1024, 512]   bf16  (f = (head, e) flattened, head-major)
  wo            [512, 1024]   bf16
  QT, KT        [512, r]      bf16  (f on partitions)   = W.T @ xT (+b)
  V (natural)   [2048, 8, 65] bf16  (s on partitions, col 64 = ones)
  scores^T      psum f32 [s_tile=128, 3, r=512] batches = KT_h.T-chunk @ QT_h
                (two heads of a pair run concurrently on PE row groups 0/64)
  expS          bf16 [128, 32, 512], exp(0.125*scores^T) in N=1536 batches
                (no max subtraction: |scores| < ~3)
  attnV         psum [65, 512] = [V_h | 1].T @ expS  (row 64 = softmax denom)
  MH^T          [512, 512-block] bf16 = attnV[0:64] * bcast(1/denom)
  out^T         [1024, 2048] f32 = wo.T @ MH^T

The QT projection for r-block r+1 is computed inside r-block r's head
loop so the PE has ready gap-filler work while ScalarE (the attention
phase bottleneck) grinds through the exps -- this keeps PE dense and the
HAM clock at 2.4 GHz.
"""
import sys
import numpy as np

try:
    import concourse.bass as bass
except ImportError:
    sys.path.insert(0, "/opt/trn_rl_repo")
    import concourse.bass as bass

import ml_dtypes
from contextlib import ExitStack

import concourse.mybir as mybir
from concourse import bacc
from concourse.tile import TileContext
from concourse.bass_utils import run_bass_kernel_spmd

BF16 = ml_dtypes.bfloat16
FP32 = mybir.dt.float32
BF = mybir.dt.bfloat16

B, S, D = 4, 2048, 1024          # batch, seq (q and kv), d_model
H, E = 8, 64                     # heads per core, head dim
F = H * E                        # 512, per-core projection width
KT = 8                           # k tiles over d_model
FT = 4                           # f tiles over F
ST = 16                          # s tiles over S
RB = 512                         # r block (matmul moving dim)
NRB = S // RB                    # 4
EB = 4                           # score-psum banks per exp batch (N=2048)

_CACHE = {}


def _build(debug=False):
    nc = bacc.Bacc()
    xT = nc.declare_dram_parameter("xT", [D, S], BF, isOutput=False)
    encT = nc.declare_dram_parameter("encT", [D, S], BF, isOutput=False)
    wq = nc.declare_dram_parameter("wq", [D, F], BF, isOutput=False)
    wk = nc.declare_dram_parameter("wk", [D, F], BF, isOutput=False)
    wv = nc.declare_dram_parameter("wv", [D, F], BF, isOutput=False)
    wo = nc.declare_dram_parameter("wo", [F, D], BF, isOutput=False)
    bq = nc.declare_dram_parameter("bq", [F, 1], FP32, isOutput=False)
    bk = nc.declare_dram_parameter("bk", [F, 1], FP32, isOutput=False)
    bvb = nc.declare_dram_parameter("bvb", [1, F], BF, isOutput=False)
    out = nc.declare_dram_parameter("out", [D, S], FP32, isOutput=True)
    if debug:
        dqt = nc.declare_dram_parameter("dqt", [128, FT, RB], BF, isOutput=True)
        dkt = nc.declare_dram_parameter("dkt", [128, FT, S], BF, isOutput=True)
        dv = nc.declare_dram_parameter("dv", [128, ST, H, E + 1], BF, isOutput=True)
        dexp = nc.declare_dram_parameter("dexp", [128, ST * 2, RB], BF, isOutput=True)
        drec = nc.declare_dram_parameter("drec", [128, RB], FP32, isOutput=True)
        dmh = nc.declare_dram_parameter("dmh", [128, FT, RB], BF, isOutput=True)

    Exp = mybir.ActivationFunctionType.Exp
    Copy = mybir.ActivationFunctionType.Copy

    with TileContext(nc) as tc, ExitStack() as ctx:
        wp = ctx.enter_context(tc.tile_pool(name="weights", bufs=1))
        ap = ctx.enter_context(tc.tile_pool(name="acts", bufs=1))
        pp = ctx.enter_context(tc.tile_pool(name="psum", bufs=2, space="PSUM"))

        # ---- weights + biases in (gpsimd SWDGE queues; sync HWDGE is kept
        # for latency-sensitive small DMAs + output) ----
        wq_s = wp.tile([128, KT, F], BF, tag="wq")
        wk_s = wp.tile([128, KT, F], BF, tag="wk")
        wv_s = wp.tile([128, KT, F], BF, tag="wv")
        for k in range(KT):
            nc.gpsimd.dma_start(out=wv_s[:, k, :], in_=wv[k * 128:(k + 1) * 128, :])
            nc.gpsimd.dma_start(out=wk_s[:, k, :], in_=wk[k * 128:(k + 1) * 128, :])
        bq_s = wp.tile([128, FT], FP32, tag="bq")
        bk_s = wp.tile([128, FT], FP32, tag="bk")
        for f in range(FT):
            nc.sync.dma_start(out=bq_s[:, f:f + 1], in_=bq[f * 128:(f + 1) * 128, :])
            nc.sync.dma_start(out=bk_s[:, f:f + 1], in_=bk[f * 128:(f + 1) * 128, :])
        bv_bf = wp.tile([1, F], BF, tag="bvb")
        ones_r = wp.tile([1, 128], BF, tag="ones")
        ones_c = wp.tile([128, 1], BF, tag="onesc")
        nc.vector.memset(ones_r, 1.0)
        nc.vector.memset(ones_c, 1.0)

        atp = ctx.enter_context(tc.tile_pool(name="attn", bufs=1))
        dsp = ctx.enter_context(tc.tile_pool(name="dscratch", bufs=2, space="DRAM"))

        def load_enc(r):
            e_r = atp.tile([128, KT, RB], BF, tag="enc", bufs=2)
            for k in range(KT):
                nc.gpsimd.dma_start(
                    out=e_r[:, k, :],
                    in_=encT[k * 128:(k + 1) * 128, r * RB:(r + 1) * RB])
            return e_r

        def load_x(r):
            x_r = atp.tile([128, KT, RB], BF, tag="xs", bufs=2)
            for k in range(KT):
                nc.gpsimd.dma_start(
                    out=x_r[:, k, :],
                    in_=xT[k * 128:(k + 1) * 128, r * RB:(r + 1) * RB])
            return x_r

        def qt_proj(x_r, f, qt_r):
            ps = pp.tile([128, RB], FP32, tag="ps_mm", bufs=1)
            for k in range(KT):
                nc.tensor.matmul(
                    ps, wq_s[:, k, f * 128:(f + 1) * 128], x_r[:, k, :],
                    start=(k == 0), stop=(k == KT - 1))
            nc.vector.tensor_scalar_add(out=qt_r[:, f, :], in0=ps,
                                        scalar1=bq_s[:, f:f + 1])

        nc.sync.dma_start(out=bv_bf, in_=bvb[:])

        # ---- V projection first (feeds attnV; ScalarE does the psum->V
        # copies so it has warm-up work while PE runs dense matmuls) ----
        kt_s = ap.tile([128, FT, S], BF, tag="kts")
        x_cur = load_x(0)
        qt_cur = atp.tile([128, FT, RB], BF, tag="qt", bufs=2)
        v_s = ap.tile([128, ST, H, E + 1], BF, tag="vs")
        nc.vector.memset(v_s[:, :, :, E:E + 1], 1.0)
        for rc in range(NRB):
            e_r = load_enc(rc)
            for sl in range(4):
                s = rc * 4 + sl
                ps = pp.tile([128, RB], FP32, tag="ps_mm", bufs=1)
                nc.tensor.matmul(ps, ones_r[:], bv_bf[:],
                                 start=True, stop=False)
                for k in range(KT):
                    nc.tensor.matmul(
                        ps, e_r[:, k, sl * 128:(sl + 1) * 128], wv_s[:, k, :],
                        start=False, stop=(k == KT - 1))
                nc.scalar.activation(
                    out=v_s[:, s, :, 0:E],
                    in_=ps.rearrange("p (h e) -> p h e", h=H),
                    func=Copy, bias=0.0, scale=1.0)

        # remaining weights (first needed mid r-block 0)
        for k in range(KT):
            nc.gpsimd.dma_start(out=wq_s[:, k, :], in_=wq[k * 128:(k + 1) * 128, :])
        wo_s = wp.tile([128, FT, D], BF, tag="wo")
        for f in range(FT):
            nc.gpsimd.dma_start(out=wo_s[:, f, :], in_=wo[f * 128:(f + 1) * 128, :])

        # ---- attention + output projection, per r block ----
        for r in range(NRB):
            rsl = slice(r * RB, (r + 1) * RB)
            mh_s = atp.tile([128, FT, RB], BF, tag="mh", bufs=2)
            if r + 1 < NRB:
                x_next = load_x(r + 1)
                qt_next = atp.tile([128, FT, RB], BF, tag="qt", bufs=2)
            for hp in range(FT):  # heads 2hp (partitions 0-63), 2hp+1 (64-127)
                if r == 0:
                    # KT projection for f=hp (enc streamed) + QT(0) f=hp,
                    # just ahead of the first scores that need them
                    for rk in range(NRB):
                        e_r = load_enc(rk)
                        ps = pp.tile([128, RB], FP32, tag="ps_mm", bufs=1)
                        for k in range(KT):
                            nc.tensor.matmul(
                                ps, wk_s[:, k, hp * 128:(hp + 1) * 128],
                                e_r[:, k, :],
                                start=(k == 0), stop=(k == KT - 1))
                        nc.vector.tensor_scalar_add(
                            out=kt_s[:, hp, rk * RB:(rk + 1) * RB], in0=ps,
                            scalar1=bk_s[:, hp:hp + 1])
                    qt_proj(x_cur, hp, qt_cur)
                expS = atp.tile([128, ST * 2, RB], BF, tag="expS", bufs=2)
                # scores^T + exp, in EB-bank psum batches over the 32
                # (s_tile, head01) slices
                j = 0
                while j < ST * 2:
                    bn = min(EB, ST * 2 - j)
                    psB = pp.tile([128, EB, RB], FP32, tag="ps_sc", bufs=1)
                    for jj in range(bn):
                        s, hh = (j + jj) // 2, (j + jj) % 2
                        ssl = slice(s * 128, (s + 1) * 128)
                        pr = slice(hh * 64, hh * 64 + 64)
                        nc.tensor.matmul(
                            psB[:, jj, :], kt_s[pr, hp, ssl], qt_cur[pr, hp, :],
                            start=True, stop=True, tile_position=(hh * 64, 0))
                    nc.scalar.activation(out=expS[:, j:j + bn, :],
                                         in_=psB[:, 0:bn, :], func=Exp, scale=0.125)
                    j += bn
                if debug and r == 0 and hp == 0:
                    nc.sync.dma_start(out=dexp[:], in_=expS[:])
                # QT projection for the next r block: PE gap-filler while
                # ScalarE works through the exps of this head pair.
                if r + 1 < NRB:
                    qt_proj(x_next, hp, qt_next)
                for hh in range(2):
                    h = 2 * hp + hh
                    po = pp.tile([128, RB], FP32, tag="ps_o", bufs=2)
                    for s in range(ST):
                        nc.tensor.matmul(
                            po[0:E + 1, :], v_s[:, s, h, :], expS[:, 2 * s + hh, :],
                            start=(s == 0), stop=(s == ST - 1))
                    rec = atp.tile([128, RB], FP32, tag="rec", bufs=2)
                    dsc = dsp.tile([1, RB], FP32, tag="dsc", bufs=4)
                    nc.vector.tensor_copy(out=rec[E:E + 1, :], in_=po[E:E + 1, :])
                    nc.sync.dma_start(out=dsc[:], in_=rec[E:E + 1, :])
                    nc.sync.dma_start(out=rec[0:64, :],
                                      in_=dsc[:].to_broadcast([64, RB]))
                    nc.vector.reciprocal_approx_fast(out=rec[0:64, :],
                                                     in_=rec[0:64, :])
                    if debug and r == 0 and hp == 0 and hh == 0:
                        nc.sync.dma_start(out=drec[:], in_=rec[:])
                    if hh == 0:
                        nc.vector.tensor_mul(
                            out=mh_s[0:64, hp, :], in0=po[0:E, :], in1=rec[0:64, :])
                    else:
                        mtmp = atp.tile([64, RB], BF, tag="mtmp", bufs=2)
                        nc.vector.tensor_mul(out=mtmp, in0=po[0:E, :],
                                             in1=rec[0:64, :])
                        nc.sync.dma_start(out=mh_s[64:128, hp, :], in_=mtmp)
            if debug and r == 0:
                nc.sync.dma_start(out=dmh[:], in_=mh_s[:])
            for dt in range(8):
                pso = pp.tile([128, RB], FP32, tag="ps_mm", bufs=1)
                for f in range(FT):
                    nc.tensor.matmul(
                        pso, wo_s[:, f, dt * 128:(dt + 1) * 128], mh_s[:, f, :],
                        start=(f == 0), stop=(f == FT - 1))
                osb = atp.tile([128, RB], FP32, tag="osb", bufs=3)
                nc.vector.tensor_copy(out=osb, in_=pso)
                nc.sync.dma_start(out=out[dt * 128:(dt + 1) * 128, rsl], in_=osb)
            if r + 1 < NRB:
                x_cur, qt_cur = x_next, qt_next

        if debug:
            nc.sync.dma_start(out=dv[:], in_=v_s[:])

    nc.finalize()
    return nc


def _prep_in_maps(x, enc, Wq, bq, Wk, bk, Wv, bv, Wo):
    def bfc(a):
        return np.ascontiguousarray(a.astype(BF16))

    in_maps = []
    for c in range(8):
        b, g = c // 2, c % 2
        hs = slice(8 * g, 8 * g + 8)
        in_maps.append({
            "xT": bfc(x[b].T),
            "encT": bfc(enc[b].T),
            "wq": bfc(np.transpose(Wq[hs], (1, 0, 2)).reshape(D, F)),
            "wk": bfc(np.transpose(Wk[hs], (1, 0, 2)).reshape(D, F)),
            "wv": bfc(np.transpose(Wv[hs], (1, 0, 2)).reshape(D, F)),
            "wo": bfc(Wo[F * g:F * (g + 1)]),
            "bq": np.ascontiguousarray(bq[hs].reshape(F, 1), dtype=np.float32),
            "bk": np.ascontiguousarray(bk[hs].reshape(F, 1), dtype=np.float32),
            "bvb": bfc(bv[hs].reshape(1, F)),
        })
    return in_maps


def run(inputs, trace=False, debug=False):
    x = np.asarray(inputs["x"], np.float32)
    enc = np.asarray(inputs["encoder_output"], np.float32)
    Wq = np.asarray(inputs["Wq"], np.float32)
    Wk = np.asarray(inputs["Wk"], np.float32)
    Wv = np.asarray(inputs["Wv"], np.float32)
    Wo = np.asarray(inputs["Wo"], np.float32)
    bq = np.asarray(inputs["bq"], np.float32)
    bk = np.asarray(inputs["bk"], np.float32)
    bv = np.asarray(inputs["bv"], np.float32)
    bo = np.asarray(inputs["bo"], np.float32)

    key = "nc_dbg" if debug else "nc"
    if key not in _CACHE:
        _CACHE[key] = _build(debug=debug)
    nc = _CACHE[key]
    in_maps = _prep_in_maps(x, enc, Wq, bq, Wk, bk, Wv, bv, Wo)
    res = run_bass_kernel_spmd(nc, in_maps, core_ids=list(range(8)), trace=trace)
    out = np.zeros((B, S, D), np.float32)
    for b in range(B):
        out[b] = (np.asarray(res.results[2 * b]["out"], np.float32)
                  + np.asarray(res.results[2 * b + 1]["out"], np.float32)).T + bo
    return out, res


def kernel(**inputs):
    out, _ = run(inputs, trace=False)
    return out
